# revision 7
# baseline (speedup 1.0000x reference)
"""Per-sample 256-bin histogram -> broadcast [B,256,256], Trainium2 Bass kernel.

Input : x int32 [64, 786432], values in [0, 256)
Output: f32 [64, 256, 256] where out[b, i, j] = count(x[b, :] == i)

Sharding: pure data parallel, 8 rows per core across 8 NeuronCores.

Per-core algorithm (cumulative-nibble decomposition + grouped outer
products):
  J[a, l]    = sum_n [x_n < 16(a+1)] * [x_n & 15 == l]   (cumulative in a)
  hist[16a+l] = J[a, l] - J[a-1, l]
  - ACT engine casts x int32 -> int16 (off the DVE critical path).
  - DVE builds 16 cumulative h-planes with single-op is_lt STRAIGHT from
    x16 (no h extraction), plus one &15 extraction and 16 is_equal
    l-planes; all mask writes are int16-in/bf16-out at 4x DVE mode.
    A few l-planes can be offloaded to GPSIMD (K_GPS).
  - Masks are stored [P, G=8, 16, C]: eight element-groups share each
    matmul. PE accumulates [128,128] PSUM outer products where the eight
    diagonal [16,16] blocks are valid per-group cumulative histograms.
    1024 elements per matmul instruction. Two PSUM accumulators
    round-robin.
  - Epilogue per row: accumulators+diag blocks -> J [16,16], difference
    along a, partition-reshape to [128,2], broadcast multiply, write out.
  Counts are integer-exact in f32 (cumulative counts < 2^24).
"""

import os
import sys

import numpy as np

sys.path.insert(0, "/opt/trn_rl_repo")

B = 64
N = 786432
NCORES = 8
ROWS_PER_CORE = B // NCORES
LEVELS = 256
P = 128

T = int(os.environ.get("K_T", "1024"))  # columns per tile (masks are 64*T B/partition)
G = 8  # element groups per matmul
C = T // G  # matmul columns per tile
TILES = N // (P * T)
assert TILES * P * T == N and C * G == T

NACC = int(os.environ.get("K_NACC", "2"))
NGPS = int(os.environ.get("K_GPS", "0"))  # l-planes offloaded to GPSIMD

_cache = {}


def _build_program(rows=None):
    import concourse.bacc as bacc
    from concourse import mybir
    from concourse import tile

    alu = mybir.AluOpType
    dt = mybir.dt

    rows = ROWS_PER_CORE if rows is None else rows

    nc = bacc.Bacc(
        "TRN2",
        target_bir_lowering=False,
        debug=False,
        num_devices=NCORES,
    )
    x_dram = nc.dram_tensor("x", [rows, N], dt.int32, kind="ExternalInput")
    out_dram = nc.dram_tensor(
        "out", [rows, LEVELS, LEVELS], dt.float32, kind="ExternalOutput"
    )

    xv = x_dram.ap().rearrange("r (t p f) -> r t p f", p=P, f=T)
    ov = out_dram.ap()

    with tile.TileContext(nc) as tc:
        with (
            tc.tile_pool(name="xin", bufs=2) as xpool,
            tc.tile_pool(name="x16", bufs=2) as x16pool,
            tc.tile_pool(name="hl", bufs=2) as hlpool,
            tc.tile_pool(name="mask", bufs=2) as mpool,
            tc.tile_pool(name="acc", bufs=2, space="PSUM") as ppool,
            tc.tile_pool(name="epi", bufs=2) as epool,
            tc.tile_pool(name="const", bufs=1) as cpool,
        ):
            ones_t = cpool.tile([P, LEVELS], dt.float32)
            nc.vector.memset(ones_t[:], 1.0)

            for r in range(rows):
                psums = [
                    ppool.tile([P, P], dt.float32, tag=f"ps{k}", name=f"ps{k}")
                    for k in range(NACC)
                ]
                for t in range(TILES):
                    x32 = xpool.tile([P, T], dt.int32, tag="x32")
                    qs = T // 4
                    for q in range(4):
                        nc.sync.dma_start(
                            out=x32[:, q * qs : (q + 1) * qs],
                            in_=xv[r, t, :, q * qs : (q + 1) * qs],
                        )

                    x16 = x16pool.tile([P, T], dt.int16, tag="x16")
                    nc.scalar.copy(out=x16[:], in_=x32[:])
                    xg = x16[:].rearrange("p (g c) -> p g c", g=G)

                    xl = hlpool.tile([P, T], dt.int16, tag="xl")
                    nc.vector.tensor_scalar(
                        out=xl[:], in0=x16[:], scalar1=15, scalar2=None,
                        op0=alu.bitwise_and,
                    )
                    xlg = xl[:].rearrange("p (g c) -> p g c", g=G)

                    hm = mpool.tile([P, G, 16, C], dt.bfloat16, tag="hm")
                    lm = mpool.tile([P, G, 16, C], dt.bfloat16, tag="lm")
                    for a in range(16):
                        # cumulative h-plane: [x < 16(a+1)]
                        nc.vector.tensor_scalar(
                            out=hm[:, :, a, :], in0=xg,
                            scalar1=16 * (a + 1), scalar2=None,
                            op0=alu.is_lt,
                        )
                        eng = nc.gpsimd if a < NGPS else nc.vector
                        eng.tensor_scalar(
                            out=lm[:, :, a, :], in0=xlg,
                            scalar1=a, scalar2=None,
                            op0=alu.is_equal,
                        )

                    for c in range(C):
                        k = c % NACC
                        nc.tensor.matmul(
                            out=psums[k][:],
                            lhsT=hm[:, :, :, c],
                            rhs=lm[:, :, :, c],
                            start=(t == 0 and c < NACC),
                            stop=(t == TILES - 1 and c >= C - NACC),
                        )

                # --- epilogue for row r ---
                # only one PSUM operand allowed per DVE instruction
                hsum = epool.tile([P, P], dt.float32, tag="hsum")
                nc.vector.tensor_copy(out=hsum[:], in_=psums[0][:])
                for k in range(1, NACC):
                    nc.vector.tensor_tensor(
                        out=hsum[:], in0=hsum[:], in1=psums[k][:],
                        op=alu.add,
                    )

                tmp = epool.tile([16, 16, G], dt.float32, tag="tmp")
                for g in range(G):
                    nc.sync.dma_start(
                        out=tmp[:, :, g],
                        in_=hsum[16 * g : 16 * (g + 1), 16 * g : 16 * (g + 1)],
                    )
                jmat = epool.tile([16, 16], dt.float32, tag="jmat")
                nc.vector.tensor_reduce(
                    out=jmat[:], in_=tmp[:], axis=mybir.AxisListType.X,
                    op=alu.add,
                )
                # difference along a (partition dim): hist[a] = J[a] - J[a-1]
                jshift = epool.tile([16, 16], dt.float32, tag="jshift")
                nc.vector.memset(jshift[0:1, :], 0.0)
                nc.sync.dma_start(out=jshift[1:16, :], in_=jmat[0:15, :])
                hist16 = epool.tile([16, 16], dt.float32, tag="h16")
                nc.vector.tensor_tensor(
                    out=hist16[:], in0=jmat[:], in1=jshift[:],
                    op=alu.subtract,
                )

                histcol = epool.tile([P, 2], dt.float32, tag="hcol")
                nc.sync.dma_start(out=histcol[:, 0:1], in_=hist16[0:8, :])
                nc.sync.dma_start(out=histcol[:, 1:2], in_=hist16[8:16, :])

                for half in range(2):
                    bt = epool.tile([P, LEVELS], dt.float32, tag="bt")
                    nc.vector.tensor_scalar(
                        out=bt[:], in0=ones_t[:],
                        scalar1=histcol[:, half : half + 1],
                        scalar2=None, op0=alu.mult,
                    )
                    nc.sync.dma_start(
                        out=ov[r, half * P : (half + 1) * P, :], in_=bt[:]
                    )

    nc.compile()
    return nc


def _get_program(rows=None):
    key = ("nc", rows)
    if key not in _cache:
        _cache[key] = _build_program(rows)
    return _cache[key]


def kernel(x: np.ndarray) -> np.ndarray:
    from concourse.bass_utils import run_bass_kernel_spmd

    x = np.ascontiguousarray(np.asarray(x), dtype=np.int32)
    assert x.shape == (B, N), x.shape

    nc = _get_program()
    in_maps = [
        {"x": x[c * ROWS_PER_CORE : (c + 1) * ROWS_PER_CORE]} for c in range(NCORES)
    ]
    res = run_bass_kernel_spmd(nc, in_maps, core_ids=list(range(NCORES)))
    out = np.concatenate([res.results[c]["out"] for c in range(NCORES)], axis=0)
    return out.astype(np.float32)


# revision 9
# speedup vs baseline: 3.0560x; 3.0560x over previous
"""Per-sample 256-bin histogram -> broadcast [B,256,256], Trainium2 Bass kernel.

Input : x int32 [64, 786432], values in [0, 256)
Output: f32 [64, 256, 256] where out[b, i, j] = count(x[b, :] == i)

Sharding: pure data parallel, 8 rows per core across 8 NeuronCores.

Per-core algorithm (cumulative-nibble decomposition + grouped outer
products):
  J[a, l]    = sum_n [x_n < 16(a+1)] * [x_n & 15 == l]   (cumulative in a)
  hist[16a+l] = J[a, l] - J[a-1, l]
  - ACT engine casts x int32 -> int16 (off the DVE critical path).
  - DVE builds 16 cumulative h-planes with single-op is_lt STRAIGHT from
    x16 (no h extraction), plus one &15 extraction and 16 is_equal
    l-planes; all mask writes are int16-in/bf16-out at 4x DVE mode.
    A few l-planes can be offloaded to GPSIMD (K_GPS).
  - Masks are stored [P, G=8, 16, C]: eight element-groups share each
    matmul. PE accumulates [128,128] PSUM outer products where the eight
    diagonal [16,16] blocks are valid per-group cumulative histograms.
    1024 elements per matmul instruction. Two PSUM accumulators
    round-robin.
  - Epilogue per row: accumulators+diag blocks -> J [16,16], difference
    along a, partition-reshape to [128,2], broadcast multiply, write out.
  Counts are integer-exact in f32 (cumulative counts < 2^24).
"""

import os
import sys

import numpy as np

sys.path.insert(0, "/opt/trn_rl_repo")

B = 64
N = 786432
NCORES = 8
ROWS_PER_CORE = B // NCORES
LEVELS = 256
P = 128

T = int(os.environ.get("K_T", "1024"))  # columns per tile (masks are 64*T B/partition)
G = 8  # element groups per matmul
C = T // G  # matmul columns per tile
TILES = N // (P * T)
assert TILES * P * T == N and C * G == T

NACC = int(os.environ.get("K_NACC", "2"))
NGPS = int(os.environ.get("K_GPS", "0"))  # l-planes offloaded to GPSIMD

_cache = {}


def _build_program(rows=None):
    import concourse.bacc as bacc
    from concourse import mybir
    from concourse import tile

    alu = mybir.AluOpType
    dt = mybir.dt

    rows = ROWS_PER_CORE if rows is None else rows

    nc = bacc.Bacc(
        "TRN2",
        target_bir_lowering=False,
        debug=False,
        num_devices=NCORES,
    )
    x_dram = nc.dram_tensor("x", [rows, N], dt.int32, kind="ExternalInput")
    out_dram = nc.dram_tensor(
        "out", [rows, LEVELS, LEVELS], dt.float32, kind="ExternalOutput"
    )

    xv = x_dram.ap().rearrange("r (t p f) -> r t p f", p=P, f=T)
    ov = out_dram.ap()

    with tile.TileContext(nc) as tc:
        with (
            tc.tile_pool(name="xin", bufs=2) as xpool,
            tc.tile_pool(name="x16", bufs=2) as x16pool,
            tc.tile_pool(name="hl", bufs=2) as hlpool,
            tc.tile_pool(name="mask", bufs=2) as mpool,
            tc.tile_pool(name="acc", bufs=2, space="PSUM") as ppool,
            tc.tile_pool(name="epi", bufs=2) as epool,
            tc.tile_pool(name="const", bufs=1) as cpool,
        ):
            ones_t = cpool.tile([P, LEVELS], dt.float32)
            nc.vector.memset(ones_t[:], 1.0)

            for r in range(rows):
                psums = [
                    ppool.tile([P, P], dt.float32, tag=f"ps{k}", name=f"ps{k}")
                    for k in range(NACC)
                ]
                for t in range(TILES):
                    x32 = xpool.tile([P, T], dt.int32, tag="x32")
                    qs = T // 4
                    for q in range(4):
                        nc.sync.dma_start(
                            out=x32[:, q * qs : (q + 1) * qs],
                            in_=xv[r, t, :, q * qs : (q + 1) * qs],
                        )

                    x16 = x16pool.tile([P, T], dt.int16, tag="x16")
                    nc.scalar.copy(out=x16[:], in_=x32[:])
                    # element (p, i) -> group g = i % G, column c = i // G
                    # (histogram is permutation-invariant, so the group
                    # interleave is free; it makes matmul operands contiguous)
                    xg = x16[:].rearrange("p (c g) -> p c g", g=G)

                    xl = hlpool.tile([P, T], dt.int16, tag="xl")
                    nc.vector.tensor_scalar(
                        out=xl[:], in0=x16[:], scalar1=15, scalar2=None,
                        op0=alu.bitwise_and,
                    )
                    xlg = xl[:].rearrange("p (c g) -> p c g", g=G)

                    # [P, C, 16, G]: per matmul column c the [16, G] block is
                    # one contiguous 256B run per partition (fast PE fetch)
                    hm = mpool.tile([P, C, 16, G], dt.bfloat16, tag="hm")
                    lm = mpool.tile([P, C, 16, G], dt.bfloat16, tag="lm")
                    for a in range(16):
                        # cumulative h-plane: [x < 16(a+1)]
                        nc.vector.tensor_scalar(
                            out=hm[:, :, a, :], in0=xg,
                            scalar1=16 * (a + 1), scalar2=None,
                            op0=alu.is_lt,
                        )
                        eng = nc.gpsimd if a < NGPS else nc.vector
                        eng.tensor_scalar(
                            out=lm[:, :, a, :], in0=xlg,
                            scalar1=a, scalar2=None,
                            op0=alu.is_equal,
                        )

                    for c in range(C):
                        k = c % NACC
                        nc.tensor.matmul(
                            out=psums[k][:],
                            lhsT=hm[:, c, :, :],
                            rhs=lm[:, c, :, :],
                            start=(t == 0 and c < NACC),
                            stop=(t == TILES - 1 and c >= C - NACC),
                        )

                # --- epilogue for row r ---
                # only one PSUM operand allowed per DVE instruction
                hsum = epool.tile([P, P], dt.float32, tag="hsum")
                nc.vector.tensor_copy(out=hsum[:], in_=psums[0][:])
                for k in range(1, NACC):
                    nc.vector.tensor_tensor(
                        out=hsum[:], in0=hsum[:], in1=psums[k][:],
                        op=alu.add,
                    )

                # valid block for group g lives at psum[a*G+g, l*G+g]
                hv = hsum[:].rearrange("(a gi) (l gj) -> a gi l gj", gi=G, gj=G)
                tmp = epool.tile([16, 16, G], dt.float32, tag="tmp")
                for g in range(G):
                    nc.sync.dma_start(out=tmp[:, :, g], in_=hv[:, g, :, g])
                jmat = epool.tile([16, 16], dt.float32, tag="jmat")
                nc.vector.tensor_reduce(
                    out=jmat[:], in_=tmp[:], axis=mybir.AxisListType.X,
                    op=alu.add,
                )
                # difference along a (partition dim): hist[a] = J[a] - J[a-1]
                jshift = epool.tile([16, 16], dt.float32, tag="jshift")
                nc.vector.memset(jshift[0:1, :], 0.0)
                nc.sync.dma_start(out=jshift[1:16, :], in_=jmat[0:15, :])
                hist16 = epool.tile([16, 16], dt.float32, tag="h16")
                nc.vector.tensor_tensor(
                    out=hist16[:], in0=jmat[:], in1=jshift[:],
                    op=alu.subtract,
                )

                histcol = epool.tile([P, 2], dt.float32, tag="hcol")
                nc.sync.dma_start(out=histcol[:, 0:1], in_=hist16[0:8, :])
                nc.sync.dma_start(out=histcol[:, 1:2], in_=hist16[8:16, :])

                for half in range(2):
                    bt = epool.tile([P, LEVELS], dt.float32, tag="bt")
                    nc.vector.tensor_scalar(
                        out=bt[:], in0=ones_t[:],
                        scalar1=histcol[:, half : half + 1],
                        scalar2=None, op0=alu.mult,
                    )
                    nc.sync.dma_start(
                        out=ov[r, half * P : (half + 1) * P, :], in_=bt[:]
                    )

    nc.compile()
    return nc


def _get_program(rows=None):
    key = ("nc", rows)
    if key not in _cache:
        _cache[key] = _build_program(rows)
    return _cache[key]


def kernel(x: np.ndarray) -> np.ndarray:
    from concourse.bass_utils import run_bass_kernel_spmd

    x = np.ascontiguousarray(np.asarray(x), dtype=np.int32)
    assert x.shape == (B, N), x.shape

    nc = _get_program()
    in_maps = [
        {"x": x[c * ROWS_PER_CORE : (c + 1) * ROWS_PER_CORE]} for c in range(NCORES)
    ]
    res = run_bass_kernel_spmd(nc, in_maps, core_ids=list(range(NCORES)))
    out = np.concatenate([res.results[c]["out"] for c in range(NCORES)], axis=0)
    return out.astype(np.float32)


# revision 15
# speedup vs baseline: 3.4058x; 1.1144x over previous
"""Per-sample 256-bin histogram -> broadcast [B,256,256], Trainium2 Bass kernel.

Input : x int32 [64, 786432], values in [0, 256)
Output: f32 [64, 256, 256] where out[b, i, j] = count(x[b, :] == i)

Sharding: pure data parallel, 8 rows per core across 8 NeuronCores.

Per-core algorithm (cumulative-nibble decomposition + grouped outer
products):
  J[a, l]    = sum_n [x_n < 16(a+1)] * [x_n & 15 == l]   (cumulative in a)
  hist[16a+l] = J[a, l] - J[a-1, l]
  - ACT engine casts x int32 -> int16 (off the DVE critical path).
  - DVE builds 16 cumulative h-planes with single-op is_lt STRAIGHT from
    x16 (no h extraction), plus one &15 extraction and 16 is_equal
    l-planes; all mask writes are int16-in/bf16-out at 4x DVE mode.
    A few l-planes can be offloaded to GPSIMD (K_GPS).
  - Masks are stored [P, G=8, 16, C]: eight element-groups share each
    matmul. PE accumulates [128,128] PSUM outer products where the eight
    diagonal [16,16] blocks are valid per-group cumulative histograms.
    1024 elements per matmul instruction. Two PSUM accumulators
    round-robin.
  - Epilogue per row: accumulators+diag blocks -> J [16,16], difference
    along a, partition-reshape to [128,2], broadcast multiply, write out.
  Counts are integer-exact in f32 (cumulative counts < 2^24).
"""

import os
import sys

import numpy as np

sys.path.insert(0, "/opt/trn_rl_repo")

B = 64
N = 786432
NCORES = 8
ROWS_PER_CORE = B // NCORES
LEVELS = 256
P = 128

T = int(os.environ.get("K_T", "1024"))  # columns per tile (masks are 64*T B/partition)
G = 8  # element groups per matmul
C = T // G  # matmul columns per tile
TILES = N // (P * T)
assert TILES * P * T == N and C * G == T

NACC = int(os.environ.get("K_NACC", "1"))
NACT = int(os.environ.get("K_ACT", "3"))  # l-planes offloaded to ACT (square+relu)

_cache = {}


def _build_program(rows=None):
    import concourse.bacc as bacc
    from concourse import mybir
    from concourse import tile

    alu = mybir.AluOpType
    dt = mybir.dt

    rows = ROWS_PER_CORE if rows is None else rows

    nc = bacc.Bacc(
        "TRN2",
        target_bir_lowering=False,
        debug=False,
        num_devices=NCORES,
    )
    x_dram = nc.dram_tensor("x", [rows, N], dt.int32, kind="ExternalInput")
    out_dram = nc.dram_tensor(
        "out", [rows, LEVELS, LEVELS], dt.float32, kind="ExternalOutput"
    )

    xv = x_dram.ap().rearrange("r (t p f) -> r t p f", p=P, f=T)
    ov = out_dram.ap()

    with tile.TileContext(nc) as tc:
        with (
            tc.tile_pool(name="xin", bufs=2) as xpool,
            tc.tile_pool(name="x16", bufs=2) as x16pool,
            tc.tile_pool(name="hl", bufs=2) as hlpool,
            tc.tile_pool(name="mask", bufs=2) as mpool,
            tc.tile_pool(name="acc", bufs=2, space="PSUM") as ppool,
            tc.tile_pool(name="epi", bufs=2) as epool,
            tc.tile_pool(name="const", bufs=1) as cpool,
        ):
            ones_t = cpool.tile([P, LEVELS], dt.float32)
            nc.vector.memset(ones_t[:], 1.0)
            # per-partition bias constants for ACT-engine mask planes
            bias_one = cpool.tile([P, 1], dt.float32)
            nc.vector.memset(bias_one[:], 1.0)
            bias_neg = cpool.tile([P, NACT if NACT else 1], dt.float32)
            for i in range(NACT):
                nc.vector.memset(bias_neg[:, i : i + 1], -float(16 - NACT + i))

            for r in range(rows):
                psums = [
                    ppool.tile([P, P], dt.float32, tag=f"ps{k}", name=f"ps{k}")
                    for k in range(NACC)
                ]
                for t in range(TILES):
                    x32 = xpool.tile([P, T], dt.int32, tag="x32")
                    qs = T // 4
                    for q in range(4):
                        nc.sync.dma_start(
                            out=x32[:, q * qs : (q + 1) * qs],
                            in_=xv[r, t, :, q * qs : (q + 1) * qs],
                        )

                    x16 = x16pool.tile([P, T], dt.int16, tag="x16")
                    nc.scalar.copy(out=x16[:], in_=x32[:])
                    # element (p, i) -> group g = i % G, column c = i // G
                    # (histogram is permutation-invariant, so the group
                    # interleave is free; it makes matmul operands contiguous)
                    xg = x16[:].rearrange("p (c g) -> p c g", g=G)

                    xl = hlpool.tile([P, T], dt.int16, tag="xl")
                    nc.vector.tensor_scalar(
                        out=xl[:], in0=x16[:], scalar1=15, scalar2=None,
                        op0=alu.bitwise_and,
                    )
                    xlg = xl[:].rearrange("p (c g) -> p c g", g=G)

                    # [P, C, 16, G]: per matmul column c the [16, G] block is
                    # one contiguous 256B run per partition (fast PE fetch)
                    hm = mpool.tile([P, C, 16, G], dt.bfloat16, tag="hm")
                    lm = mpool.tile([P, C, 16, G], dt.bfloat16, tag="lm")
                    for a in range(16):
                        # cumulative h-plane: [x < 16(a+1)]
                        nc.vector.tensor_scalar(
                            out=hm[:, :, a, :], in0=xg,
                            scalar1=16 * (a + 1), scalar2=None,
                            op0=alu.is_lt,
                        )
                        if a < 16 - NACT:
                            nc.vector.tensor_scalar(
                                out=lm[:, :, a, :], in0=xlg,
                                scalar1=a, scalar2=None,
                                op0=alu.is_equal,
                            )
                        else:
                            # ACT engine: [xl == a] as relu(1 - (xl - a)^2)
                            sq = hlpool.tile([P, T], dt.bfloat16, tag="sq")
                            i = a - (16 - NACT)
                            nc.scalar.activation(
                                sq[:], xl[:],
                                mybir.ActivationFunctionType.Square,
                                bias=bias_neg[:, i : i + 1], scale=1.0,
                            )
                            sqg = sq[:].rearrange("p (c g) -> p c g", g=G)
                            nc.scalar.activation(
                                lm[:, :, a, :], sqg,
                                mybir.ActivationFunctionType.Relu,
                                bias=bias_one[:], scale=-1.0,
                            )

                    for c in range(C):
                        k = c % NACC
                        nc.tensor.matmul(
                            out=psums[k][:],
                            lhsT=hm[:, c, :, :],
                            rhs=lm[:, c, :, :],
                            start=(t == 0 and c < NACC),
                            stop=(t == TILES - 1 and c >= C - NACC),
                        )

                # --- epilogue for row r ---
                # only one PSUM operand allowed per DVE instruction; use the
                # ACT engine for the PSUM drain to keep DVE free
                hsum = epool.tile([P, P], dt.float32, tag="hsum")
                nc.scalar.copy(out=hsum[:], in_=psums[0][:])
                for k in range(1, NACC):
                    nc.vector.tensor_tensor(
                        out=hsum[:], in0=hsum[:], in1=psums[k][:],
                        op=alu.add,
                    )

                # valid block for group g lives at psum[a*G+g, l*G+g]
                hv = hsum[:].rearrange("(a gi) (l gj) -> a gi l gj", gi=G, gj=G)
                tmp = epool.tile([16, 16, G], dt.float32, tag="tmp")
                for g in range(G):
                    nc.sync.dma_start(out=tmp[:, :, g], in_=hv[:, g, :, g])
                jmat = epool.tile([16, 16], dt.float32, tag="jmat")
                nc.vector.tensor_reduce(
                    out=jmat[:], in_=tmp[:], axis=mybir.AxisListType.X,
                    op=alu.add,
                )
                # difference along a (partition dim): hist[a] = J[a] - J[a-1]
                jshift = epool.tile([16, 16], dt.float32, tag="jshift")
                nc.vector.memset(jshift[0:1, :], 0.0)
                nc.sync.dma_start(out=jshift[1:16, :], in_=jmat[0:15, :])
                hist16 = epool.tile([16, 16], dt.float32, tag="h16")
                nc.vector.tensor_tensor(
                    out=hist16[:], in0=jmat[:], in1=jshift[:],
                    op=alu.subtract,
                )

                histcol = epool.tile([P, 2], dt.float32, tag="hcol")
                nc.sync.dma_start(out=histcol[:, 0:1], in_=hist16[0:8, :])
                nc.sync.dma_start(out=histcol[:, 1:2], in_=hist16[8:16, :])

                for half in range(2):
                    bt = epool.tile([P, LEVELS], dt.float32, tag="bt")
                    nc.scalar.mul(bt[:], ones_t[:], histcol[:, half : half + 1])
                    nc.sync.dma_start(
                        out=ov[r, half * P : (half + 1) * P, :], in_=bt[:]
                    )

    nc.compile()
    return nc


def _get_program(rows=None):
    key = ("nc", rows)
    if key not in _cache:
        _cache[key] = _build_program(rows)
    return _cache[key]


def kernel(x: np.ndarray) -> np.ndarray:
    from concourse.bass_utils import run_bass_kernel_spmd

    x = np.ascontiguousarray(np.asarray(x), dtype=np.int32)
    assert x.shape == (B, N), x.shape

    nc = _get_program()
    in_maps = [
        {"x": x[c * ROWS_PER_CORE : (c + 1) * ROWS_PER_CORE]} for c in range(NCORES)
    ]
    res = run_bass_kernel_spmd(nc, in_maps, core_ids=list(range(NCORES)))
    out = np.concatenate([res.results[c]["out"] for c in range(NCORES)], axis=0)
    return out.astype(np.float32)
